# revision 15
# baseline (speedup 1.0000x reference)
"""Trainium2 Bass kernel for the DAN classifier (gather + segment-mean + MLP).

Full computation:
    gathered = embeddings[docs]                    # [B, L, D]
    avg = gathered.sum(1) / doc_lens[:, None]      # [B, D]
    out = relu(relu(avg @ W1 + b1) @ W2 + b2) @ W3 + b3   # [B, C]

Reformulation: the fused gather + segment-sum is a sparse matmul
    doc_sums[b, :] = sum_v CNT[v, b] * embeddings[v, :]
where CNT[v, b] counts occurrences of vocab v in doc b (host-built with one
bincount). Vocab-shard the table across 8 cores: core k streams its 12,500
table rows sequentially at full DMA bandwidth, multiplies on the PE against
its CNT shard, producing partial doc sums for ALL 256 docs, then one
ReduceScatter(add) hands each core the finished sums for its own 32 docs.

Precision: table rows stream as plain fp16 (measured end-to-end rel err
~1.6e-4, far under the 2e-2 gate). Counts are small ints (max ~3), exact in
fp8e4m3, halving the count-matrix traffic. Accumulation is fp32 in PSUM;
segment-mean and the 3-layer MLP run in fp32.

Layouts are host-prearranged so every DMA descriptor is a contiguous
multi-KB per-partition line: th[g, p, c, :] = fp16 table row g*1280+c*128+p,
ct[g, p, c, :] = fp8 counts for the same row. Per-core traffic: 7.68 MB
table + 3.28 MB counts ~= 31 us at 360 GB/s; PE work ~= 30 us of fp16
matmul. Stream DMAs issue from both HWDGE sequencers (SP + ACT) and MLP
constants load via SWDGE (GpSimd) so sequencer time never gates the stream.
"""

import numpy as np

# Problem shapes (hardcoded per contract).
V, D = 100000, 300
B, L = 256, 1000
H, C = 512, 5
NCORES = 8
BC = B // NCORES            # docs per core = 32
VSH = V // NCORES           # vocab rows per core = 12500
NCHK = 100                  # 128-row chunks per core (12800 rows, zero-padded)
VSHP = NCHK * 128           # padded shard rows = 12800
CCH = 10                    # chunks per DMA group
NGRP = NCHK // CCH          # 10 DMA groups
DCH = 100                   # D split for transposes / W1 chunks (3 x 100)

_CACHE = {}


def _build_nc():
    import concourse.bass as bass
    import concourse.bacc as bacc
    import concourse.mybir as mybir
    import concourse.tile as tile

    dt = mybir.dt
    f32 = dt.float32
    fp16 = dt.float16
    fp8 = dt.float8e4

    nc = bacc.Bacc("TRN2", target_bir_lowering=False, debug=False, num_devices=NCORES)

    th_d = nc.dram_tensor("th", [NGRP, 128, CCH, D], fp16, kind="ExternalInput")
    ct_d = nc.dram_tensor("ct", [NGRP, 128, CCH, B], fp8, kind="ExternalInput")
    invl_d = nc.dram_tensor("invl", [BC, 1], f32, kind="ExternalInput")
    w1_d = nc.dram_tensor("w1", [DCH, 3 * H], f32, kind="ExternalInput")
    w2_d = nc.dram_tensor("w2", [128, 4 * H], f32, kind="ExternalInput")
    w3_d = nc.dram_tensor("w3", [128, 4 * C], f32, kind="ExternalInput")
    b1_d = nc.dram_tensor("b1", [128, 4], f32, kind="ExternalInput")
    b2_d = nc.dram_tensor("b2", [128, 4], f32, kind="ExternalInput")
    b3_d = nc.dram_tensor("b3", [1, C], f32, kind="ExternalInput")
    ones_d = nc.dram_tensor("ones", [1, BC], f32, kind="ExternalInput")
    ident_d = nc.dram_tensor("ident", [BC, BC], f32, kind="ExternalInput")
    out_d = nc.dram_tensor("out", [BC, C], f32, kind="ExternalOutput")

    cc_in = nc.dram_tensor("cc_in", [B, D], f32)
    cc_out = nc.dram_tensor("cc_out", [BC, D], f32)

    relu = mybir.ActivationFunctionType.Relu

    with tile.TileContext(nc) as tc:
        with (
            tc.tile_pool(name="const", bufs=1) as cp,
            tc.tile_pool(name="tstream", bufs=3) as tp,
            tc.tile_pool(name="cstream", bufs=3) as ctp,
            tc.tile_pool(name="work", bufs=1) as wp,
            tc.tile_pool(name="psacc", bufs=1, space="PSUM") as pp,
            tc.tile_pool(name="psmlp", bufs=3, space="PSUM") as pp2,
        ):
            # MLP constants via SWDGE (GpSimd) — keeps the HWDGE sequencers
            # free for stream DMAs at startup.
            invl_sb = cp.tile([BC, 1], f32)
            nc.gpsimd.dma_start(out=invl_sb[:], in_=invl_d[:])
            w1_sb = cp.tile([DCH, 3 * H], f32)
            nc.gpsimd.dma_start(out=w1_sb[:], in_=w1_d[:])
            w2_sb = cp.tile([128, 4 * H], f32)
            nc.gpsimd.dma_start(out=w2_sb[:], in_=w2_d[:])
            w3_sb = cp.tile([128, 4 * C], f32)
            nc.gpsimd.dma_start(out=w3_sb[:], in_=w3_d[:])
            b1_sb = cp.tile([128, 4], f32)
            nc.gpsimd.dma_start(out=b1_sb[:], in_=b1_d[:])
            b2_sb = cp.tile([128, 4], f32)
            nc.gpsimd.dma_start(out=b2_sb[:], in_=b2_d[:])
            b3_sb = cp.tile([1, C], f32)
            nc.gpsimd.dma_start(out=b3_sb[:], in_=b3_d[:])
            ones_sb = cp.tile([1, BC], f32)
            nc.gpsimd.dma_start(out=ones_sb[:], in_=ones_d[:])
            ident_sb = cp.tile([BC, BC], f32)
            nc.gpsimd.dma_start(out=ident_sb[:], in_=ident_d[:])

            # Partial doc sums: docs 0:128 / 128:256.
            psA = pp.tile([128, D], f32, tag="psA")
            psB = pp.tile([128, D], f32, tag="psB")

            for g in range(NGRP):
                tt = tp.tile([128, CCH, D], fp16)
                nc.sync.dma_start(out=tt[:], in_=th_d[g])
                ct = ctp.tile([128, CCH, B], fp8)
                nc.scalar.dma_start(out=ct[:], in_=ct_d[g])
                for c in range(CCH):
                    chunk = g * CCH + c
                    st, sp_ = chunk == 0, chunk == NCHK - 1
                    nc.tensor.matmul(
                        out=psA[:], lhsT=ct[:, c, 0:128], rhs=tt[:, c, :],
                        start=st, stop=sp_,
                    )
                    nc.tensor.matmul(
                        out=psB[:], lhsT=ct[:, c, 128:256], rhs=tt[:, c, :],
                        start=st, stop=sp_,
                    )

            # Stage partials in SBUF, then push to DRAM for the collective.
            s0 = wp.tile([128, D], f32)
            nc.vector.tensor_copy(out=s0[:], in_=psA[:])
            nc.sync.dma_start(out=cc_in[0:128, :], in_=s0[:])
            s1 = wp.tile([128, D], f32)
            nc.vector.tensor_copy(out=s1[:], in_=psB[:])
            nc.scalar.dma_start(out=cc_in[128:256, :], in_=s1[:])

            # Sum partials across cores; rank k keeps docs 32k..32k+31.
            nc.gpsimd.collective_compute(
                "ReduceScatter",
                mybir.AluOpType.add,
                replica_groups=[list(range(NCORES))],
                ins=[cc_in[:]],
                outs=[cc_out[:]],
            )

            ds = wp.tile([BC, D], f32)
            nc.sync.dma_start(out=ds[:], in_=cc_out[:])

            # Mean: divide by doc length (per-partition scalar).
            avg = wp.tile([BC, D], f32)
            nc.vector.tensor_scalar_mul(avg[:], ds[:], invl_sb[:])

            # Transpose to [D, BC] in three 100-column chunks.
            avgT = wp.tile([DCH, 3 * BC], f32)
            for c3 in range(3):
                pt = pp2.tile([DCH, BC], f32, tag="mlp")
                nc.tensor.transpose(
                    out=pt[:],
                    in_=avg[:, c3 * DCH : (c3 + 1) * DCH],
                    identity=ident_sb[:],
                )
                nc.vector.tensor_copy(out=avgT[:, c3 * BC : (c3 + 1) * BC], in_=pt[:])

            # Layer 1: h1T[j] = relu(W1[:, j-chunk]^T @ avgT + b1), j over 4x128.
            h1 = wp.tile([128, 4 * BC], f32)
            for j in range(4):
                p1 = pp2.tile([128, BC], f32, tag="mlp")
                for c3 in range(3):
                    nc.tensor.matmul(
                        out=p1[:],
                        lhsT=w1_sb[:, c3 * H + 128 * j : c3 * H + 128 * j + 128],
                        rhs=avgT[:, c3 * BC : (c3 + 1) * BC],
                        start=(c3 == 0),
                        stop=(c3 == 2),
                    )
                nc.scalar.activation(
                    out=h1[:, j * BC : (j + 1) * BC],
                    in_=p1[:],
                    func=relu,
                    bias=b1_sb[:, j : j + 1],
                )

            # Layer 2: h2T[j] = relu(sum_k W2[k-chunk, j-chunk]^T @ h1T[k] + b2).
            h2 = wp.tile([128, 4 * BC], f32)
            for j in range(4):
                p2 = pp2.tile([128, BC], f32, tag="mlp")
                for k in range(4):
                    nc.tensor.matmul(
                        out=p2[:],
                        lhsT=w2_sb[:, k * H + 128 * j : k * H + 128 * j + 128],
                        rhs=h1[:, k * BC : (k + 1) * BC],
                        start=(k == 0),
                        stop=(k == 3),
                    )
                nc.scalar.activation(
                    out=h2[:, j * BC : (j + 1) * BC],
                    in_=p2[:],
                    func=relu,
                    bias=b2_sb[:, j : j + 1],
                )

            # Layer 3: out = sum_j h2T[j]^T @ W3[j-chunk] + b3 (bias via K=1 matmul).
            pout = pp2.tile([BC, C], f32, tag="mlp")
            for j in range(4):
                nc.tensor.matmul(
                    out=pout[:],
                    lhsT=h2[:, j * BC : (j + 1) * BC],
                    rhs=w3_sb[:, j * C : (j + 1) * C],
                    start=(j == 0),
                    stop=False,
                )
            nc.tensor.matmul(
                out=pout[:], lhsT=ones_sb[:], rhs=b3_sb[:], start=False, stop=True
            )

            out_sb = wp.tile([BC, C], f32)
            nc.vector.tensor_copy(out=out_sb[:], in_=pout[:])
            nc.sync.dma_start(out=out_d[:], in_=out_sb[:])

    nc.finalize()
    return nc


def _get_nc():
    if "nc" not in _CACHE:
        _CACHE["nc"] = _build_nc()
    return _CACHE["nc"]


def make_in_maps(embeddings, W1, b1, W2, b2, W3, b3, docs, doc_lens):
    """Host-side sharding: fp16 table shards + per-shard fp8 count matrices."""
    import ml_dtypes

    fp8 = ml_dtypes.float8_e4m3fn
    emb = np.asarray(embeddings, np.float32)
    docs = np.asarray(docs, np.int32)
    doc_lens = np.asarray(doc_lens, np.int32)

    # CNT[b, v] = multiplicity of vocab v in doc b.
    ids = (np.arange(B, dtype=np.int64)[:, None] * V + docs.astype(np.int64)).ravel()
    cnt_full = np.bincount(ids, minlength=B * V).reshape(B, V)

    w1 = np.ascontiguousarray(
        np.asarray(W1, np.float32).reshape(3, DCH, H).transpose(1, 0, 2).reshape(DCH, 3 * H)
    )
    w2 = np.ascontiguousarray(
        np.asarray(W2, np.float32).reshape(4, 128, H).transpose(1, 0, 2).reshape(128, 4 * H)
    )
    w3 = np.ascontiguousarray(
        np.asarray(W3, np.float32).reshape(4, 128, C).transpose(1, 0, 2).reshape(128, 4 * C)
    )
    b1p = np.ascontiguousarray(np.asarray(b1, np.float32).reshape(4, 128).T)
    b2p = np.ascontiguousarray(np.asarray(b2, np.float32).reshape(4, 128).T)
    b3r = np.ascontiguousarray(np.asarray(b3, np.float32).reshape(1, C))
    ones = np.ones((1, BC), np.float32)
    ident = np.eye(BC, dtype=np.float32)

    in_maps = []
    for core in range(NCORES):
        x = emb[core * VSH : (core + 1) * VSH]            # [12500, 300]
        hi = np.zeros((VSHP, D), np.float16)
        hi[:VSH] = x.astype(np.float16)
        # [VSHP, D] -> [NGRP, 128, CCH, D]: row g*1280 + c*128 + p -> [g, p, c]
        th = np.ascontiguousarray(
            hi.reshape(NGRP, CCH, 128, D).transpose(0, 2, 1, 3)
        )
        cs = np.zeros((VSHP, B), fp8)
        cs[:VSH] = cnt_full[:, core * VSH : (core + 1) * VSH].T.astype(fp8)
        ct = np.ascontiguousarray(
            cs.reshape(NGRP, CCH, 128, B).transpose(0, 2, 1, 3)
        )
        invl = (1.0 / doc_lens[core * BC : (core + 1) * BC].astype(np.float32)).reshape(
            BC, 1
        )
        in_maps.append(
            {
                "th": th,
                "ct": ct,
                "invl": np.ascontiguousarray(invl),
                "w1": w1,
                "w2": w2,
                "w3": w3,
                "b1": b1p,
                "b2": b2p,
                "b3": b3r,
                "ones": ones,
                "ident": ident,
            }
        )
    return in_maps


def kernel(embeddings, W1, b1, W2, b2, W3, b3, docs, doc_lens):
    from concourse.bass_utils import run_bass_kernel_spmd

    nc = _get_nc()
    in_maps = make_in_maps(embeddings, W1, b1, W2, b2, W3, b3, docs, doc_lens)
    res = run_bass_kernel_spmd(nc, in_maps, list(range(NCORES)))
    out = np.concatenate([res.results[i]["out"] for i in range(NCORES)], axis=0)
    return out.astype(np.float32)
